# revision 1
# baseline (speedup 1.0000x reference)
"""Multi-head self-attention (S=2048, B=2, D=1024, H=16) on 8 TRN2 NeuronCores.

Sharding: core c handles batch b = c//4 and head-quad g = c%4 (4 heads of 64).
Megatron-style: in_proj column-sliced, out_proj row-sliced; host sums the 8
partial outputs and adds out_proj bias.

Per-core dataflow (matmul inputs bf16, accumulation fp32):
  - host supplies x^T (D-major) activations and pre-transposed weight slices
  - qpT/kpT computed head-major (m on partitions, seq on free)
  - vp computed seq-major with an interleaved ones column per head (65-wide
    blocks) so the PV matmul also produces softmax row-sums on partition 64
  - scores^T per (head-pair, 512-query-chunk, key-tile) in a packed psum tile
    (128, 2, 512); exp on ACT reads the pair in one op
  - normalization: K=1 matmul broadcasts the row-sums, DVE divides
  - out-projection on device from attn^T; bias + cross-core reduction on host
"""

import math
from contextlib import ExitStack, nullcontext as _null_ctx

import numpy as np

S = 2048
B = 2
D = 1024
H = 16
DK = 64
HC = 4          # heads per core
M = HC * DK     # 256 head-dim columns per core
N_CORES = 8
KT = S // 128   # 16 key tiles
QQ = 4          # 512-wide query chunks

MM_DT = "bfloat16"   # dtype of matmul inputs ("bfloat16" or "float32r")

_compiled = None


def _build_program():
    import concourse.tile as tile
    from concourse import mybir, bacc

    f32 = mybir.dt.float32
    f32r = mybir.dt.float32r
    mdt = getattr(mybir.dt, MM_DT)
    EXP = mybir.ActivationFunctionType.Exp

    nc = bacc.Bacc("TRN2", target_bir_lowering=False, debug=False)

    xqT = nc.dram_tensor("xqT", [D, S], mdt, kind="ExternalInput").ap()
    xkT = nc.dram_tensor("xkT", [D, S], mdt, kind="ExternalInput").ap()
    xvT = nc.dram_tensor("xvT", [D, S], mdt, kind="ExternalInput").ap()
    wqT = nc.dram_tensor("wqT", [D, M], mdt, kind="ExternalInput").ap()
    wkT = nc.dram_tensor("wkT", [D, M], mdt, kind="ExternalInput").ap()
    wvT = nc.dram_tensor("wvT", [D, M], mdt, kind="ExternalInput").ap()
    bq = nc.dram_tensor("bq", [M], f32, kind="ExternalInput").ap()
    bk = nc.dram_tensor("bk", [M], f32, kind="ExternalInput").ap()
    bv = nc.dram_tensor("bv", [M], mdt, kind="ExternalInput").ap()
    woT = nc.dram_tensor("woT", [M, D], mdt, kind="ExternalInput").ap()
    ones32_dr = nc.dram_tensor("ones32", [1, 64], f32r, kind="ExternalInput").ap()
    out = nc.dram_tensor("out", [S, D], f32, kind="ExternalOutput").ap()

    with tile.TileContext(nc) as tc, ExitStack() as ctx:
        const_pool = ctx.enter_context(tc.tile_pool(name="const", bufs=1))
        x_pool = ctx.enter_context(tc.tile_pool(name="x", bufs=16))
        xv_pool = ctx.enter_context(tc.tile_pool(name="xv", bufs=16))
        e_pool = ctx.enter_context(tc.tile_pool(name="e", bufs=20))
        o_pool = ctx.enter_context(tc.tile_pool(name="o", bufs=2))
        r_pool = ctx.enter_context(tc.tile_pool(name="r", bufs=2))
        ps_a = ctx.enter_context(tc.tile_pool(name="ps_a", bufs=2, space="PSUM"))
        ps_b = ctx.enter_context(tc.tile_pool(name="ps_b", bufs=4, space="PSUM"))

        # ---- persistent SBUF tensors ----
        # weight slices as matmul lhsT, K-chunked: [p, kc, m]
        wq_sb = const_pool.tile([128, 8, M], mdt)
        wk_sb = const_pool.tile([128, 8, M], mdt)
        wv_sb = const_pool.tile([128, 8, M], mdt)
        for w_sb, w_dr in ((wq_sb, wqT), (wk_sb, wkT), (wv_sb, wvT)):
            nc.sync.dma_start(
                out=w_sb[:, :, :], in_=w_dr.rearrange("(kc p) m -> p kc m", p=128)
            )
        # out_proj rhs: [p, kc, j]
        wo_sb = const_pool.tile([128, 2, D], mdt)
        nc.sync.dma_start(
            out=wo_sb[:, :, :], in_=woT.rearrange("(kc p) j -> p kc j", p=128)
        )
        # per-partition biases for qpT/kpT: [p, mt]
        bq_sb = const_pool.tile([128, 2], f32)
        bk_sb = const_pool.tile([128, 2], f32)
        nc.sync.dma_start(out=bq_sb[:, :], in_=bq.rearrange("(mt p) -> p mt", p=128))
        nc.sync.dma_start(out=bk_sb[:, :], in_=bk.rearrange("(mt p) -> p mt", p=128))
        # bv as a K=1 matmul rhs row
        bv_sb = const_pool.tile([1, M], mdt)
        nc.sync.dma_start(out=bv_sb[:, :], in_=bv.rearrange("(a m) -> a m", a=1))
        ones_sb = const_pool.tile([1, 128], mdt)
        nc.vector.memset(ones_sb[:, :], 1.0)
        ones32_sb = const_pool.tile([1, 64], f32r)
        nc.sync.dma_start(out=ones32_sb[:, :], in_=ones32_dr[:, :])

        qpT = const_pool.tile([128, 2, S], mdt)   # [p, mt, s]
        kpT = const_pool.tile([128, 2, S], mdt)
        vp = const_pool.tile([128, KT, HC * 65], mdt)  # aug: 65-wide per head
        attnT = const_pool.tile([128, 2, S], mdt)

        # ones columns of the augmented V (once; head h at column h*65+64)
        nc.vector.memset(
            vp[:, :, :].rearrange("p kt (h c) -> p kt h c", c=65)[:, :, :, 64:65], 1.0
        )

        # ---- projections ----
        # x^T K-chunks stay resident (x_pool holds all 16 per tensor), so
        # each weight m-tile can be projected independently of load order.
        def load_half(x_dr, half, pool=None, eng=None):
            fs = half * 1024
            chunks = []
            for kc in range(8):
                xt = (pool or x_pool).tile([128, 1024], mdt, tag="xchunk")
                (eng or nc.sync).dma_start(
                    out=xt[:, :], in_=x_dr[kc * 128:(kc + 1) * 128, fs:fs + 1024]
                )
                chunks.append((xt, fs))
            return chunks

        def load_chunks(x_dr, pool=None, eng=None):
            return load_half(x_dr, 0, pool, eng) + load_half(x_dr, 1, pool, eng)

        def proj_half(chunks, w_sb, b_sb, p_sb, mt, half):
            fs = half * 1024
            for nch in range(2):
                ns = nch * 512
                ps = ps_b.tile([128, 512], f32, tag="ps_small", name="ps_p")
                for kc in range(8):
                    nc.tensor.matmul(
                        ps[:, :],
                        w_sb[:, kc, mt * 128:(mt + 1) * 128],
                        chunks[half * 8 + kc][0][:, ns:ns + 512],
                        start=(kc == 0),
                        stop=(kc == 7),
                    )
                nc.vector.tensor_scalar_add(
                    out=p_sb[:, mt, fs + ns:fs + ns + 512],
                    in0=ps[:, :],
                    scalar1=b_sb[:, mt:mt + 1],
                )

        def vp_group(chunks, kt):
            half, st = divmod(kt, 8)
            ps = ps_b.tile([128, 256], f32, tag="ps_small", name="ps_v")
            for kc in range(8):
                nc.tensor.matmul(
                    ps[:, 0:M],
                    chunks[half * 8 + kc][0][:, st * 128:(st + 1) * 128],
                    wv_sb[:, kc, :],
                    start=(kc == 0),
                    stop=False,
                )
            # bias via K=1 ones-row matmul
            nc.tensor.matmul(
                ps[:, 0:M],
                ones_sb[0:1, 0:128],
                bv_sb[0:1, :],
                start=False,
                stop=True,
            )
            nc.vector.tensor_copy(
                out=vp[:, kt, :].rearrange("p (h c) -> p h c", c=65)[:, :, 0:64],
                in_=ps[:, 0:M].rearrange("p (h c) -> p h c", c=64),
            )

        # interleave loads so scores for the first keys can start after just
        # the first half of xk + xq has landed, with xv staged in between so
        # the just-in-time V projection keeps pace with the PV consumers
        # mt0 projections run on freshly-streamed chunks; the x tiles are
        # then re-streamed later for the mt1 projections (cheap DMA, far off
        # the critical path) so the pool stays small and the E runway large.
        chunks_k = load_half(xkT, 0)
        proj_half(chunks_k, wk_sb, bk_sb, kpT, 0, 0)
        chunks_q = load_half(xqT, 0)
        proj_half(chunks_q, wq_sb, bq_sb, qpT, 0, 0)
        chunks_k += load_half(xkT, 1)
        proj_half(chunks_k, wk_sb, bk_sb, kpT, 0, 1)
        chunks_q += load_half(xqT, 1)
        proj_half(chunks_q, wq_sb, bq_sb, qpT, 0, 1)
        chunks_v = load_chunks(xvT, pool=xv_pool)
        chunks_k2 = load_chunks(xkT)
        for half in range(2):
            proj_half(chunks_k2, wk_sb, bk_sb, kpT, 1, half)
        chunks_q2 = load_chunks(xqT)
        for half in range(2):
            proj_half(chunks_q2, wq_sb, bq_sb, qpT, 1, half)

        # ---- attention + out-projection ----
        # The per-engine runtime schedule is static and in-order, so a
        # segment's normalization/out-projection is emitted INSIDE the next
        # segment's kt loop — its DVE-latency chain then overlaps the next
        # segment's compute instead of head-of-line blocking the PE queue.
        def flush_head(pair, qq, u, hh):
            qs = qq * 512
            rs = r_pool.tile([1, 512], f32r, tag="rs")
            with nc.allow_low_precision(reason="softmax denom"):
                nc.vector.tensor_copy(out=rs[:, :], in_=u[64:65, :])
            us = r_pool.tile([64, 512], f32, tag="us")
            nc.vector.tensor_copy(out=us[:, :], in_=u[0:64, :])
            rb = ps_b.tile([64, 512], f32, tag="ps_small")
            nc.tensor.matmul(
                rb[0:64, :], ones32_sb[0:1, 0:64], rs[0:1, :], start=True, stop=True
            )
            rbs = r_pool.tile([64, 512], f32, tag="rbs")
            nc.vector.reciprocal_approx_fast(out=rbs[:, :], in_=rb[0:64, :])
            with nc.allow_low_precision(reason="softmax normalize"):
                nc.vector.tensor_tensor(
                    out=attnT[hh * 64:hh * 64 + 64, pair, qs:qs + 512],
                    in0=us[0:64, :],
                    in1=rbs[0:64, :],
                    op=mybir.AluOpType.mult,
                )

        def outproj_stile(sg):
            ot = o_pool.tile([128, D], f32)
            for nch in range(2):
                ns = nch * 512
                po = ps_b.tile([128, 512], f32, tag="ps_small")
                for kc in range(2):
                    nc.tensor.matmul(
                        po[:, :],
                        attnT[:, kc, sg * 128:(sg + 1) * 128],
                        wo_sb[:, kc, ns:ns + 512],
                        start=(kc == 0),
                        stop=(kc == 1),
                    )
                nc.vector.tensor_copy(out=ot[:, ns:ns + 512], in_=po[:, :])
            nc.sync.dma_start(out=out[sg * 128:(sg + 1) * 128, :], in_=ot[:, :])

        pending_flush = None   # (pair, qq, u_tiles) awaiting normalization
        pending_out = []       # out-projection s-tiles ready to interleave
        for pair in range(2):
            for qq in range(QQ):
                qs = qq * 512
                u_tiles = []
                for h in (2 * pair, 2 * pair + 1):
                    u_tiles.append(
                        ps_b.tile([65, 512], f32, tag="ps_small", name=f"u_{qq}_{h}")
                    )
                for kt in range(KT):
                    ks = kt * 128
                    with tc.high_priority() if pair == 0 else _null_ctx():
                        sc = ps_a.tile([128, 2, 512], f32, tag="ps_main")
                        for hh in range(2):
                            po = hh * 64
                            nc.tensor.matmul(
                                sc[:, hh, :],
                                kpT[po:po + 64, pair, ks:ks + 128],
                                qpT[po:po + 64, pair, qs:qs + 512],
                                start=True,
                                stop=True,
                            )
                        et = e_pool.tile([128, 2, 512], mdt)
                        nc.scalar.activation(out=et[:, :, :], in_=sc[:, :, :], func=EXP)
                    if pair == 0 and qq == 0:
                        # V projection emitted just-in-time for its first consumer
                        vp_group(chunks_v, kt)
                    for hh in range(2):
                        h = 2 * pair + hh
                        nc.tensor.matmul(
                            u_tiles[hh][0:65, :],
                            vp[:, kt, h * 65:(h + 1) * 65],
                            et[:, hh, :],
                            start=(kt == 0),
                            stop=(kt == KT - 1),
                        )
                    # interleave the previous segment's epilogue
                    if pending_flush is not None and kt in (2, 4):
                        p_pair, p_qq, p_u = pending_flush
                        flush_head(p_pair, p_qq, p_u[kt // 2 - 1], kt // 2 - 1)
                        if kt == 4:
                            if p_pair == 1:
                                pending_out.extend(range(p_qq * 4, p_qq * 4 + 4))
                            pending_flush = None
                    elif pending_out and kt in (6, 9, 12, 15):
                        outproj_stile(pending_out.pop(0))
                pending_flush = (pair, qq, u_tiles)
        # tail: last segment's normalization + remaining out-projection
        p_pair, p_qq, p_u = pending_flush
        flush_head(p_pair, p_qq, p_u[0], 0)
        flush_head(p_pair, p_qq, p_u[1], 1)
        pending_out.extend(range(p_qq * 4, p_qq * 4 + 4))
        for sg in pending_out:
            outproj_stile(sg)

    nc.compile()
    return nc


def _get_compiled():
    global _compiled
    if _compiled is None:
        _compiled = _build_program()
    return _compiled


def _make_in_maps(q, k, v, in_proj_w, in_proj_b, out_proj_w):
    import ml_dtypes

    mdt_np = np.dtype(ml_dtypes.bfloat16) if MM_DT == "bfloat16" else np.float32

    def cvt(a):
        return np.ascontiguousarray(a).astype(mdt_np)

    xT = {}
    for b in range(B):
        xT[b] = (
            cvt(q[:, b, :].T),
            cvt(k[:, b, :].T),
            cvt(v[:, b, :].T),
        )
    scale = 1.0 / math.sqrt(DK)
    in_maps = []
    for c in range(N_CORES):
        b, g = divmod(c, HC)
        cols = slice(g * M, (g + 1) * M)
        in_maps.append({
            "xqT": xT[b][0],
            "xkT": xT[b][1],
            "xvT": xT[b][2],
            "wqT": cvt((in_proj_w[0 * D:1 * D][cols] * scale).T),
            "wkT": cvt(in_proj_w[1 * D:2 * D][cols].T),
            "wvT": cvt(in_proj_w[2 * D:3 * D][cols].T),
            "bq": np.ascontiguousarray(in_proj_b[0 * D:1 * D][cols] * scale),
            "bk": np.ascontiguousarray(in_proj_b[1 * D:2 * D][cols]),
            "bv": cvt(in_proj_b[2 * D:3 * D][cols]),
            "woT": cvt(out_proj_w[:, g * M:(g + 1) * M].T),
            "ones32": np.ones((1, 64), dtype=np.float32),
        })
    return in_maps


def kernel(q, k, v, in_proj_w, in_proj_b, out_proj_w, out_proj_b):
    from concourse.bass_utils import run_bass_kernel_spmd

    q = np.asarray(q, dtype=np.float32)
    k = np.asarray(k, dtype=np.float32)
    v = np.asarray(v, dtype=np.float32)
    in_proj_w = np.asarray(in_proj_w, dtype=np.float32)
    in_proj_b = np.asarray(in_proj_b, dtype=np.float32)
    out_proj_w = np.asarray(out_proj_w, dtype=np.float32)
    out_proj_b = np.asarray(out_proj_b, dtype=np.float32)

    nc = _get_compiled()
    in_maps = _make_in_maps(q, k, v, in_proj_w, in_proj_b, out_proj_w)

    res = run_bass_kernel_spmd(nc, in_maps, core_ids=list(range(N_CORES)))

    out = np.broadcast_to(out_proj_b.astype(np.float32), (S, B, D)).copy()
    for c in range(N_CORES):
        out[:, c // HC, :] += res.results[c]["out"]
    return out



# revision 10
# speedup vs baseline: 1.0385x; 1.0385x over previous
"""Multi-head self-attention (S=2048, B=2, D=1024, H=16) on 8 TRN2 NeuronCores.

Sharding: core c handles batch b = c//4 and head-quad g = c%4 (4 heads of 64).
Megatron-style: in_proj column-sliced, out_proj row-sliced; host sums the 8
partial outputs and adds out_proj bias.

Schedule (v2): the attention inner loop is ACT(exp)-paced; all other PE work
(projections, V-projection, out-projection) is woven into it as fine-grained
filler units so the tensor engine never runs a long blob that starves the
scalar engine.

  - per (pair, qq) segment, kt runs in groups of 2: [S,S] (64-row-tiled
    score pairs) then [PV x4] (128-row mode), halving PE mode switches
  - PSUM: scores ring 2x[128,2,512] (4 banks) + u ring 3x[128,512]
    per-head accumulators (3 banks) + 1 filler slot (1 bank)
  - x chunks stay resident in SBUF (no DMA re-streaming for the mt1
    projections); DMA emitted in critical-path order on the sync queue
  - v-proj bias folded into the DVE psum drain (no bias matmuls)
  - softmax normalization: DVE reciprocal of the row-sum row, GPSIMD
    partition_broadcast, DVE multiply straight out of PSUM (no broadcast
    matmul, no extra PSUM bank)
  - output stored bf16, DMA'd on the (idle) gpsimd queue
"""

import math
from contextlib import ExitStack

import numpy as np

S = 2048
B = 2
D = 1024
H = 16
DK = 64
HC = 4          # heads per core
M = HC * DK     # 256 head-dim columns per core
N_CORES = 8
KT = S // 128   # 16 key tiles
QQ = 4          # 512-wide query chunks

MM_DT = "bfloat16"

_compiled = None


def _build_program():
    import concourse.tile as tile
    from concourse import mybir, bacc

    f32 = mybir.dt.float32
    mdt = getattr(mybir.dt, MM_DT)
    EXP = mybir.ActivationFunctionType.Exp
    ADD = mybir.AluOpType.add
    MULT = mybir.AluOpType.mult

    nc = bacc.Bacc("TRN2", target_bir_lowering=False, debug=False)

    xqT = nc.dram_tensor("xqT", [D, S], mdt, kind="ExternalInput").ap()
    xkT = nc.dram_tensor("xkT", [D, S], mdt, kind="ExternalInput").ap()
    xvT = nc.dram_tensor("xvT", [D, S], mdt, kind="ExternalInput").ap()
    wqT = nc.dram_tensor("wqT", [D, M], mdt, kind="ExternalInput").ap()
    wkT = nc.dram_tensor("wkT", [D, M], mdt, kind="ExternalInput").ap()
    wvT = nc.dram_tensor("wvT", [D, M], mdt, kind="ExternalInput").ap()
    bq = nc.dram_tensor("bq", [M], f32, kind="ExternalInput").ap()
    bk = nc.dram_tensor("bk", [M], f32, kind="ExternalInput").ap()
    bv = nc.dram_tensor("bv", [M], mdt, kind="ExternalInput").ap()
    woT = nc.dram_tensor("woT", [M, D], mdt, kind="ExternalInput").ap()
    out = nc.dram_tensor("out", [S, D], mdt, kind="ExternalOutput").ap()

    with tile.TileContext(nc) as tc, ExitStack() as ctx:
        const_pool = ctx.enter_context(tc.tile_pool(name="const", bufs=1))
        xk_pool = ctx.enter_context(tc.tile_pool(name="xk", bufs=16))
        xq_pool = ctx.enter_context(tc.tile_pool(name="xq", bufs=16))
        xv_pool = ctx.enter_context(tc.tile_pool(name="xv", bufs=16))
        e_pool = ctx.enter_context(tc.tile_pool(name="e", bufs=8))
        ot_pool = ctx.enter_context(tc.tile_pool(name="ot", bufs=2))
        r_pool = ctx.enter_context(tc.tile_pool(name="r", bufs=2))
        ps_sc = ctx.enter_context(tc.tile_pool(name="ps_sc", bufs=2, space="PSUM"))
        ps_u = ctx.enter_context(tc.tile_pool(name="ps_u", bufs=3, space="PSUM"))
        ps_f = ctx.enter_context(tc.tile_pool(name="ps_f", bufs=1, space="PSUM"))

        # ---- persistent SBUF tensors ----
        wq_sb = const_pool.tile([128, 8, M], mdt)
        wk_sb = const_pool.tile([128, 8, M], mdt)
        wv_sb = const_pool.tile([128, 8, M], mdt)
        wo_sb = const_pool.tile([128, 2, D], mdt)
        bq_sb = const_pool.tile([128, 2], f32)
        bk_sb = const_pool.tile([128, 2], f32)
        bv_sb = const_pool.tile([1, M], mdt)
        ones_sb = const_pool.tile([1, 128], mdt)
        nc.vector.memset(ones_sb[:, :], 1.0)

        qpT = const_pool.tile([128, 2, S], mdt)   # [p, mt, s]
        kpT = const_pool.tile([128, 2, S], mdt)
        vp = const_pool.tile([128, KT, HC * 65], mdt)  # aug: 65-wide per head
        attnT = const_pool.tile([128, 2, S], mdt)

        # ones columns of the augmented V (head h's ones at column h*65+64)
        nc.vector.memset(
            vp[:, :, :].rearrange("p kt (h c) -> p kt h c", c=65)[:, :, :, 64:65], 1.0
        )

        # ---- DMA emission, critical-path order ----
        # sync queue: wk, xk-h0 | wq, xq-h0 | wv, xv-h0 | xk-h1 | xv-h1 |
        #             xq-h1 | wo.  gpsimd queue: biases.
        def load_half(x_dr, half, pool):
            fs = half * 1024
            chunks = []
            for kc in range(8):
                xt = pool.tile([128, 1024], mdt, tag="xchunk")
                nc.sync.dma_start(
                    out=xt[:, :], in_=x_dr[kc * 128:(kc + 1) * 128, fs:fs + 1024]
                )
                chunks.append(xt)
            return chunks

        def load_w(w_sb, w_dr):
            nc.sync.dma_start(
                out=w_sb[:, :, :], in_=w_dr.rearrange("(kc p) m -> p kc m", p=128)
            )

        load_w(wk_sb, wkT)
        chunks_k = load_half(xkT, 0, xk_pool)
        load_w(wq_sb, wqT)
        chunks_q = load_half(xqT, 0, xq_pool)
        load_w(wv_sb, wvT)
        chunks_v = load_half(xvT, 0, xv_pool)
        chunks_k += load_half(xkT, 1, xk_pool)
        chunks_v += load_half(xvT, 1, xv_pool)
        chunks_q += load_half(xqT, 1, xq_pool)
        nc.sync.dma_start(
            out=wo_sb[:, :, :], in_=woT.rearrange("(kc p) j -> p kc j", p=128)
        )
        nc.gpsimd.dma_start(out=bq_sb[:, :], in_=bq.rearrange("(mt p) -> p mt", p=128))
        nc.gpsimd.dma_start(out=bk_sb[:, :], in_=bk.rearrange("(mt p) -> p mt", p=128))
        nc.gpsimd.dma_start(out=bv_sb[:, :], in_=bv.rearrange("(a m) -> a m", a=1))

        # ---- filler units (generators; each yield ~= 2 matmuls or a drain) ----
        def gen_proj(pT, w_sb, b_sb, mt, half, nch, chunks):
            fs, ns = half * 1024, nch * 512
            ps = ps_f.tile([128, 512], f32, tag="f", name=f"pp{mt}{half}{nch}")
            for kc in range(8):
                nc.tensor.matmul(
                    ps[:, :],
                    w_sb[:, kc, mt * 128:(mt + 1) * 128],
                    chunks[half * 8 + kc][:, ns:ns + 512],
                    start=(kc == 0),
                    stop=(kc == 7),
                )
                if kc % 2 == 1:
                    yield
            nc.vector.tensor_scalar_add(
                out=pT[:, mt, fs + ns:fs + ns + 512],
                in0=ps[:, :],
                scalar1=b_sb[:, mt:mt + 1],
            )
            yield

        def gen_vp(kt):
            half, st = divmod(kt, 8)
            ps = ps_f.tile([128, 512], f32, tag="f", name=f"pv{kt}")
            for kc in range(8):
                nc.tensor.matmul(
                    ps[:, 0:M],
                    chunks_v[half * 8 + kc][:, st * 128:(st + 1) * 128],
                    wv_sb[:, kc, :],
                    start=(kc == 0),
                    stop=False,
                )
                if kc % 2 == 1:
                    yield
            # bias via K=1 ones-row matmul into the same accumulation
            nc.tensor.matmul(
                ps[:, 0:M],
                ones_sb[0:1, 0:128],
                bv_sb[0:1, :],
                start=False,
                stop=True,
            )
            nc.vector.tensor_copy(
                out=vp[:, kt, :].rearrange("p (h c) -> p h c", c=65)[:, :, 0:64],
                in_=ps[:, 0:M].rearrange("p (h c) -> p h c", c=64),
            )
            yield

        def gen_outproj(sg):
            ot = ot_pool.tile([128, D], mdt, tag="ot", name=f"ot{sg}")
            for nch in range(2):
                ns = nch * 512
                po = ps_f.tile([128, 512], f32, tag="f", name=f"po{sg}{nch}")
                for kc in range(2):
                    nc.tensor.matmul(
                        po[:, :],
                        attnT[:, kc, sg * 128:(sg + 1) * 128],
                        wo_sb[:, kc, ns:ns + 512],
                        start=(kc == 0),
                        stop=(kc == 1),
                    )
                with nc.allow_low_precision(reason="bf16 output"):
                    nc.vector.tensor_copy(out=ot[:, ns:ns + 512], in_=po[:, :])
                yield
            nc.gpsimd.dma_start(out=out[sg * 128:(sg + 1) * 128, :], in_=ot[:, :])
            yield

        def run_full(gen):
            for _ in gen:
                pass

        # ---- flush: normalize a head's accumulated PV into attnT ----
        def emit_flush(pair, qq, hh, u):
            qs = qq * 512
            rs = r_pool.tile([1, 512], f32, tag="rs")
            nc.vector.tensor_copy(out=rs[:, :], in_=u[64:65, :])
            rbi = r_pool.tile([1, 512], f32, tag="rbi")
            with nc.allow_low_precision(reason="softmax denom"):
                nc.vector.reciprocal_approx_fast(out=rbi[:, :], in_=rs[0:1, :])
            us = r_pool.tile([64, 512], f32, tag="us")
            nc.vector.tensor_copy(out=us[:, :], in_=u[0:64, :])
            rbb = r_pool.tile([64, 512], f32, tag="rbb")
            nc.gpsimd.partition_broadcast(rbb[0:64, :], rbi[0:1, :], channels=64)
            with nc.allow_low_precision(reason="softmax normalize"):
                nc.vector.tensor_tensor(
                    out=attnT[hh * 64:hh * 64 + 64, pair, qs:qs + 512],
                    in0=us[:, :],
                    in1=rbb[0:64, :],
                    op=MULT,
                )

        # ---- preamble projections (needed before the first scores) ----
        run_full(gen_proj(kpT, wk_sb, bk_sb, 0, 0, 0, chunks_k))
        run_full(gen_proj(kpT, wk_sb, bk_sb, 0, 0, 1, chunks_k))
        run_full(gen_proj(qpT, wq_sb, bq_sb, 0, 0, 0, chunks_q))

        # ---- attention master loop ----
        # work deque of filler generators, consumed in order
        work = []
        work_budget_steps = 2

        def advance(n):
            while n > 0 and work:
                try:
                    next(work[0])
                    n -= 1
                except StopIteration:
                    work.pop(0)

        def S_pair(pair, qq, kt, sc):
            qs = qq * 512
            ks = kt * 128
            for hh in range(2):
                po = hh * 64
                nc.tensor.matmul(
                    sc[:, hh, :],
                    kpT[po:po + 64, pair, ks:ks + 128],
                    qpT[po:po + 64, pair, qs:qs + 512],
                    start=True,
                    stop=True,
                )

        def PV(pair, kt, hh, u, et):
            h = 2 * pair + hh
            nc.tensor.matmul(
                u[0:65, :],
                vp[:, kt, h * 65:(h + 1) * 65],
                et[:, hh, :],
                start=(kt == 0),
                stop=(kt == KT - 1),
            )

        SEGS = [(p, q) for p in range(2) for q in range(4)]
        pending_flush = None   # (pair, qq, [u_h0, u_h1])

        for si, (pair, qq) in enumerate(SEGS):
            # add this segment's filler units to the deque
            if si == 1:
                work.append(gen_proj(qpT, wq_sb, bq_sb, 0, 1, 0, chunks_q))  # c2
                work.append(gen_proj(kpT, wk_sb, bk_sb, 1, 0, 0, chunks_k))
                work.append(gen_proj(kpT, wk_sb, bk_sb, 1, 0, 1, chunks_k))
            elif si == 2:
                work.append(gen_proj(qpT, wq_sb, bq_sb, 0, 1, 1, chunks_q))  # c3
                work.append(gen_proj(kpT, wk_sb, bk_sb, 1, 1, 0, chunks_k))
                work.append(gen_proj(kpT, wk_sb, bk_sb, 1, 1, 1, chunks_k))
            elif si == 3:
                work.append(gen_proj(qpT, wq_sb, bq_sb, 1, 0, 0, chunks_q))
                work.append(gen_proj(qpT, wq_sb, bq_sb, 1, 0, 1, chunks_q))
            elif si == 4:
                work.append(gen_proj(qpT, wq_sb, bq_sb, 1, 1, 0, chunks_q))
            elif si == 5:
                work.append(gen_proj(qpT, wq_sb, bq_sb, 1, 1, 1, chunks_q))
            if pair == 1 and qq >= 1:
                for sg in range((qq - 1) * 4, qq * 4):
                    work.append(gen_outproj(sg))

            u_tiles = [None, None]
            et_tiles = {}
            for g in range(KT // 2):
                kts = (2 * g, 2 * g + 1)
                sc = ps_sc.tile([128, 2, 512], f32, tag="sc", name=f"sc{si}_{g}")
                S_pair(pair, qq, kts[0], sc)
                et0 = e_pool.tile([128, 2, 512], mdt, tag="et")
                nc.scalar.activation(out=et0[:, :, :], in_=sc[:, :, :], func=EXP)
                et_tiles[kts[0]] = et0

                sc2 = ps_sc.tile([128, 2, 512], f32, tag="sc", name=f"sc{si}_{g}b")
                S_pair(pair, qq, kts[1], sc2)
                et1 = e_pool.tile([128, 2, 512], mdt, tag="et")
                nc.scalar.activation(out=et1[:, :, :], in_=sc2[:, :, :], func=EXP)
                et_tiles[kts[1]] = et1

                if si == 0:
                    # V-projection just-in-time: vp(kt) ready one group
                    # before its first PV consumer
                    run_full(gen_vp(kts[0]))
                    run_full(gen_vp(kts[1]))

                # PVs of the previous group
                if g > 0:
                    for kt in (2 * g - 2, 2 * g - 1):
                        if u_tiles[0] is None:
                            u_tiles[0] = ps_u.tile(
                                [128, 512], f32, tag="u", name=f"u{si}_0"
                            )
                            u_tiles[1] = ps_u.tile(
                                [128, 512], f32, tag="u", name=f"u{si}_1"
                            )
                        PV(pair, kt, 0, u_tiles[0], et_tiles[kt])
                        PV(pair, kt, 1, u_tiles[1], et_tiles[kt])
                        del et_tiles[kt]

                # interleave the previous segment's flush early in this one
                if g == 0 and pending_flush is not None:
                    p_pair, p_qq, p_u = pending_flush
                    emit_flush(p_pair, p_qq, 0, p_u[0])
                    emit_flush(p_pair, p_qq, 1, p_u[1])
                    pending_flush = None

                if si == 0:
                    if g in (2, 3):
                        # kpT mt0 half1 needed by S(kt8) at g4
                        run_full(
                            gen_proj(kpT, wk_sb, bk_sb, 0, 1, g - 2, chunks_k)
                        )
                    elif g == 6:
                        # qpT mt0 c1 needed by segment 1
                        run_full(
                            gen_proj(qpT, wq_sb, bq_sb, 0, 0, 1, chunks_q)
                        )
                else:
                    advance(work_budget_steps if pair == 0 else 3)

            # segment epilogue: last group's PVs
            for kt in (KT - 2, KT - 1):
                PV(pair, kt, 0, u_tiles[0], et_tiles[kt])
                PV(pair, kt, 1, u_tiles[1], et_tiles[kt])
                del et_tiles[kt]
            pending_flush = (pair, qq, u_tiles)

        # tail: final segment's flush + last out-projection chunk
        p_pair, p_qq, p_u = pending_flush
        emit_flush(p_pair, p_qq, 0, p_u[0])
        emit_flush(p_pair, p_qq, 1, p_u[1])
        for sg in range(12, 16):
            work.append(gen_outproj(sg))
        while work:
            advance(1000)

    nc.compile()
    return nc


def _get_compiled():
    global _compiled
    if _compiled is None:
        _compiled = _build_program()
    return _compiled


def _make_in_maps(q, k, v, in_proj_w, in_proj_b, out_proj_w):
    import ml_dtypes

    mdt_np = np.dtype(ml_dtypes.bfloat16) if MM_DT == "bfloat16" else np.float32

    def cvt(a):
        return np.ascontiguousarray(a).astype(mdt_np)

    xT = {}
    for b in range(B):
        xT[b] = (
            cvt(q[:, b, :].T),
            cvt(k[:, b, :].T),
            cvt(v[:, b, :].T),
        )
    scale = 1.0 / math.sqrt(DK)
    in_maps = []
    for c in range(N_CORES):
        b, g = divmod(c, HC)
        cols = slice(g * M, (g + 1) * M)
        in_maps.append({
            "xqT": xT[b][0],
            "xkT": xT[b][1],
            "xvT": xT[b][2],
            "wqT": cvt((in_proj_w[0 * D:1 * D][cols] * scale).T),
            "wkT": cvt(in_proj_w[1 * D:2 * D][cols].T),
            "wvT": cvt(in_proj_w[2 * D:3 * D][cols].T),
            "bq": np.ascontiguousarray(in_proj_b[0 * D:1 * D][cols] * scale),
            "bk": np.ascontiguousarray(in_proj_b[1 * D:2 * D][cols]),
            "bv": cvt(in_proj_b[2 * D:3 * D][cols]),
            "woT": cvt(out_proj_w[:, g * M:(g + 1) * M].T),
        })
    return in_maps


def kernel(q, k, v, in_proj_w, in_proj_b, out_proj_w, out_proj_b):
    from concourse.bass_utils import run_bass_kernel_spmd

    q = np.asarray(q, dtype=np.float32)
    k = np.asarray(k, dtype=np.float32)
    v = np.asarray(v, dtype=np.float32)
    in_proj_w = np.asarray(in_proj_w, dtype=np.float32)
    in_proj_b = np.asarray(in_proj_b, dtype=np.float32)
    out_proj_w = np.asarray(out_proj_w, dtype=np.float32)
    out_proj_b = np.asarray(out_proj_b, dtype=np.float32)

    nc = _get_compiled()
    in_maps = _make_in_maps(q, k, v, in_proj_w, in_proj_b, out_proj_w)

    res = run_bass_kernel_spmd(nc, in_maps, core_ids=list(range(N_CORES)))

    out = np.broadcast_to(out_proj_b.astype(np.float32), (S, B, D)).copy()
    for c in range(N_CORES):
        out[:, c // HC, :] += res.results[c]["out"].astype(np.float32)
    return out
